# revision 1
# baseline (speedup 1.0000x reference)
"""Trainium2 Bass kernel: out = softmax(gelu_tanh(x @ W^T), axis=-1) + bias.

Full shapes: x [8192, 4096] f32, weight [4096, 4096] f32, bias [4096] f32.
Sharding: data-parallel over rows of x across 8 NeuronCores (1024 rows/core);
weight and bias replicated. Matmul runs in bf16 on the PE array with fp32
PSUM accumulation; gelu is computed with the exact tanh-approx constants of
the reference via DVE + ACT(Tanh), and softmax needs no max-subtraction
because gelu output is bounded in [-0.17, ~3.5] so exp cannot overflow.

Per-core loop structure (MC=1024 rows):
  split rows into G=2 groups of 512; for each group, stream weight n-tiles
  (512 cols) with the x-group resident in SBUF; accumulate 32 k-matmuls into
  PSUM per (m-tile, n-tile); fuse exp(gelu(v)) into the PSUM->SBUF epilogue
  with per-row sums accumulated by the ACT engine; normalize + bias-add with
  one fused DVE op per tile, then DMA out.

Measured on trn2 (8 cores): ~495 us HW exec, PE busy ~447 us (bf16 matmul
roofline for 2*8192*4096*4096 flops split 8 ways = 437 us), max error
1.1e-3 relative to absmax. tanh+exp share one ACT table set (exp_and_others)
so there is exactly one ACT_TABLE_LOAD. An fp8e4m3 DoubleRow variant
(fp8=True, weight pre-scaled x64) measures ~337 us but its error
(1.2e-2 of absmax) fails strict allclose thresholds, so bf16 is default.
"""

import sys

if "/opt/trn_rl_repo" not in sys.path:
    sys.path.insert(0, "/opt/trn_rl_repo")

import ml_dtypes
import numpy as np

import concourse.bass as bass
import concourse.tile as tile
from concourse import bacc, mybir
from concourse.bass_utils import run_bass_kernel_spmd

P = 128
GELU_A = 0.044715
GELU_C = 0.7978845608

# Full-problem constants (hardcoded; harness calls kernel() with these shapes)
FULL_M, FULL_K, FULL_N = 8192, 4096, 4096
NCORES = 8
MC = FULL_M // NCORES  # rows per core
G = 2                  # row groups per core
NT = 512               # n tile (columns per weight tile / psum)


W_SCALE = 64.0  # fp8 only: weight values ~U(-1/64,1/64) sit at e4m3's min-normal
                # boundary; scale into [-1,1] for the matmul, undo via ACT scale.


def build_nc(MC=MC, K=FULL_K, N=FULL_N, G=G, NT=NT, fp8=False):
    """Emit the per-core Bass program. Each core computes MC rows."""
    KO = K // P            # k subtiles of 128
    MG = MC // G           # rows per m-group
    MT = MG // P           # 128-row m-tiles per group
    NTILES = N // NT
    f32 = mybir.dt.float32
    bf16 = mybir.dt.bfloat16
    in_dt = mybir.dt.float8e4 if fp8 else bf16
    kstep = 2 if fp8 else 1  # DoubleRow contracts 2 k-subtiles per matmul
    inv_scale = 1.0 / W_SCALE if fp8 else 1.0

    nc = bacc.Bacc("TRN2", target_bir_lowering=False, debug=False)
    xt = nc.dram_tensor("xt", [G, P, KO, MG], in_dt, kind="ExternalInput").ap()
    wt = nc.dram_tensor("wt", [NTILES, P, KO, NT], in_dt, kind="ExternalInput").ap()
    bias = nc.dram_tensor("bias", [P, N], f32, kind="ExternalInput").ap()
    out = nc.dram_tensor("out", [P, MC // P, N], f32, kind="ExternalOutput").ap()

    with tile.TileContext(nc) as tc:
        # k-chunking of the streaming DMAs: matmuls can start as soon as the
        # first chunk lands (Tile tracks slice-level deps), instead of waiting
        # for a full 4MB tile. x gets one spare slot so the next group's first
        # chunk prefetches while the current group is still computing.
        XCH = 4 if KO % 4 == 0 else 1   # x chunks per group
        KX = KO // XCH
        WCH = 4 if KO % 4 == 0 else 1   # w chunks per n-tile
        KW = KO // WCH
        with (
            tc.tile_pool(name="const", bufs=1) as const_pool,
            tc.tile_pool(name="x", bufs=XCH + 1) as x_pool,
            tc.tile_pool(name="w", bufs=2) as w_pool,
            tc.tile_pool(name="probs", bufs=1) as probs_pool,
            tc.tile_pool(name="tmp", bufs=2) as tmp_pool,
            tc.tile_pool(name="stat", bufs=2) as stat_pool,
            tc.tile_pool(name="stage", bufs=4) as stage_pool,
            tc.tile_pool(name="psum", bufs=8, space="PSUM") as psum_pool,
        ):
            bias_t = const_pool.tile([P, N], f32)

            for g in range(G):
                # Emit x and first-w chunks interleaved in k-ascending order so
                # the DMA queues deliver them in consumption order; bias (only
                # needed by the first normalize, ~50us in) goes after.
                xcs = []
                w0 = w_pool.tile([P, KO, NT], in_dt, tag="w_t")
                for c in range(XCH):
                    nc.gpsimd.dma_start(
                        w0[:, c * KW : (c + 1) * KW, :],
                        wt[0, :, c * KW : (c + 1) * KW, :],
                    )
                    xc = x_pool.tile([P, KX, MG], in_dt, tag="xc")
                    nc.gpsimd.dma_start(xc[:], xt[g, :, c * KX : (c + 1) * KX, :])
                    xcs.append(xc)
                if g == 0:
                    nc.gpsimd.dma_start(bias_t[:], bias[:])
                probs = probs_pool.tile([P, MT, N], bf16)
                sums = stat_pool.tile([P, MT * NTILES], f32, tag="sums")
                for j in range(NTILES):
                    if j == 0:
                        w_t = w0
                    else:
                        w_t = w_pool.tile([P, KO, NT], in_dt, tag="w_t")
                        for c in range(WCH):
                            nc.gpsimd.dma_start(
                                w_t[:, c * KW : (c + 1) * KW, :],
                                wt[j, :, c * KW : (c + 1) * KW, :],
                            )
                    for i in range(MT):
                        ps = psum_pool.tile([P, NT], f32)
                        for k in range(0, KO, kstep):
                            if kstep == 2:
                                kc, kl = k // KX, k % KX
                                nc.tensor.matmul(
                                    ps[:],
                                    xcs[kc][:, kl : kl + 2, i * P : (i + 1) * P],
                                    w_t[:, k : k + 2, :],
                                    start=(k == 0),
                                    stop=(k == KO - 2),
                                    perf_mode=mybir.MatmulPerfMode.DoubleRow,
                                )
                            else:
                                nc.tensor.matmul(
                                    ps[:],
                                    xcs[k // KX][:, k % KX, i * P : (i + 1) * P],
                                    w_t[:, k, :],
                                    start=(k == 0),
                                    stop=(k == KO - 1),
                                )
                        # p = exp(gelu(v)) with gelu = 0.5*v*(1+tanh(C*(v+A*v^3)))
                        # v^2 via ACT Square straight from PSUM (Square is a
                        # filler fn in every ACT table set -> no table reload);
                        # every later op reads PSUM at most once, as HW requires.
                        v2 = tmp_pool.tile([P, NT], f32, tag="v2")
                        nc.scalar.activation(
                            v2[:], ps[:], mybir.ActivationFunctionType.Square,
                            bias=0.0, scale=inv_scale,
                        )
                        t1 = tmp_pool.tile([P, NT], f32, tag="t1")
                        nc.vector.tensor_scalar(
                            t1[:], v2[:], GELU_A * inv_scale, inv_scale,
                            mybir.AluOpType.mult, mybir.AluOpType.add,
                        )
                        t2 = tmp_pool.tile([P, NT], f32, tag="t2")
                        nc.vector.tensor_mul(t2[:], ps[:], t1[:])
                        th = tmp_pool.tile([P, NT], f32, tag="th")
                        nc.scalar.activation(
                            th[:], t2[:], mybir.ActivationFunctionType.Tanh,
                            bias=0.0, scale=GELU_C,
                        )
                        g2 = tmp_pool.tile([P, NT], f32, tag="g2")
                        nc.vector.scalar_tensor_tensor(
                            g2[:], th[:], 1.0, ps[:],
                            mybir.AluOpType.add, mybir.AluOpType.mult,
                        )
                        sidx = i * NTILES + j
                        nc.scalar.activation(
                            probs[:, i, j * NT : (j + 1) * NT], g2[:],
                            mybir.ActivationFunctionType.Exp,
                            bias=0.0, scale=0.5 * inv_scale,
                            accum_out=sums[:, sidx : sidx + 1],
                        )
                ssum = stat_pool.tile([P, MT], f32, tag="ssum")
                recips = stat_pool.tile([P, MT], f32, tag="recips")
                for i in range(MT):
                    nc.vector.reduce_sum(
                        ssum[:, i : i + 1],
                        sums[:, i * NTILES : (i + 1) * NTILES],
                        axis=mybir.AxisListType.X,
                    )
                    nc.vector.reciprocal(recips[:, i : i + 1], ssum[:, i : i + 1])
                    for j in range(NTILES):
                        st = stage_pool.tile([P, NT], f32)
                        nc.vector.scalar_tensor_tensor(
                            st[:],
                            probs[:, i, j * NT : (j + 1) * NT],
                            recips[:, i : i + 1],
                            bias_t[:, j * NT : (j + 1) * NT],
                            mybir.AluOpType.mult,
                            mybir.AluOpType.add,
                        )
                        nc.gpsimd.dma_start(out[:, g * MT + i, j * NT : (j + 1) * NT], st[:])
    nc.compile()
    return nc


def pack_inputs(x, weight, bias, MC=MC, G=G, NT=NT, fp8=False):
    """Host-side shard + pack into the DMA-friendly layouts the kernel expects."""
    M, K = x.shape
    N = weight.shape[0]
    KO = K // P
    MG = MC // G
    NTILES = N // NT
    ncores = M // MC
    in_np = mybir.dt.np(mybir.dt.float8e4) if fp8 else ml_dtypes.bfloat16
    w_src = weight * W_SCALE if fp8 else weight
    # wt[j, p, ko, n] = weight[j*NT+n, ko*P+p]
    wt = np.ascontiguousarray(
        w_src.astype(in_np).reshape(NTILES, NT, KO, P).transpose(0, 3, 2, 1)
    )
    bias_b = np.ascontiguousarray(
        np.broadcast_to(bias.astype(np.float32)[None, :], (P, N))
    )
    in_maps = []
    for c in range(ncores):
        xs = x[c * MC : (c + 1) * MC].astype(in_np)
        # xt[g, p, ko, m] = x_core[g*MG+m, ko*P+p]
        xtc = np.ascontiguousarray(xs.reshape(G, MG, KO, P).transpose(0, 3, 2, 1))
        in_maps.append({"xt": xtc, "wt": wt, "bias": bias_b})
    return in_maps


def unpack_outputs(results, MC=MC, N=FULL_N):
    outs = []
    for res in results:
        o = np.asarray(res["out"])  # [P, MC//P, N]
        outs.append(o.transpose(1, 0, 2).reshape(MC, N))
    return np.concatenate(outs, axis=0)


USE_FP8 = False

_CACHE = {}


def _get_nc(fp8=USE_FP8):
    key = ("nc", fp8)
    if key not in _CACHE:
        _CACHE[key] = build_nc(fp8=fp8)
    return _CACHE[key]


def _ensure_trace_env():
    """The agent image's antenv lacks axon_hooks, so NTFF tracing silently
    degrades. Register the ctypes-based hook ourselves, and neuter the S3
    artifact upload (no bucket access here)."""
    try:
        from antenv.axon_hooks import get_axon_ntff_profile_hook  # noqa: F401
    except ImportError:
        import types

        import antenv
        from trn_agent_boot.trn_boot import _ntff_profile_via_ctypes

        mod = types.ModuleType("antenv.axon_hooks")
        state = {"hook": _ntff_profile_via_ctypes("/opt/axon/libaxon_pjrt.so")}
        mod.set_axon_ntff_profile_hook = lambda h: state.__setitem__("hook", h)
        mod.get_axon_ntff_profile_hook = lambda: state["hook"]
        sys.modules["antenv.axon_hooks"] = mod
        antenv.axon_hooks = mod
    import concourse.bass_utils as bu

    bu.upload_artifacts = lambda tmpdir: f"local://{tmpdir}"


def kernel(x, weight, bias, trace=False, fp8=USE_FP8):
    if trace:
        _ensure_trace_env()
    nc = _get_nc(fp8)
    in_maps = pack_inputs(
        np.asarray(x, dtype=np.float32),
        np.asarray(weight, dtype=np.float32),
        np.asarray(bias, dtype=np.float32),
        fp8=fp8,
    )
    res = run_bass_kernel_spmd(nc, in_maps, core_ids=list(range(NCORES)), trace=trace)
    out = unpack_outputs(res.results)
    if trace:
        return out, res
    return out



# revision 6
# speedup vs baseline: 1.1671x; 1.1671x over previous
"""Trainium2 Bass kernel: out = softmax(gelu_tanh(x @ W^T), axis=-1) + bias.

Full shapes: x [8192, 4096] f32, weight [4096, 4096] f32, bias [4096] f32.
Sharding: data-parallel over rows of x across 8 NeuronCores (1024 rows/core);
weight and bias replicated.

Matmul runs in fp8e4m3 DoubleRow (2 fp8 MACs/cell/cycle = 157 TF/s peak;
measured stream spacing 216ns per 512-col 256-k matmul = 155 TF/s). Weight
values ~U(-1/64,1/64) sit at e4m3's min-normal boundary, so they are
pre-scaled x64 into [-1,1]; the scale is undone inside the fused epilogue.
End-to-end scale-relative error vs the f32 reference is ~1.2e-2 (gate 2e-2):
fp8 quantization of both operands gives ~5% rms per-term error which largely
cancels through the row softmax.

Structure per core (MC=1024 rows = 8 m-tiles of 128):
  - W [4096,4096] fp8 is kept FULLY SBUF-resident (131KB/partition),
    DMA'd once in 512-col slabs.
  - Phase 1 (m-tiles 0..2): j-outer over n-slabs, consuming each W slab as
    it lands (needs ~200GB/s of the 358GB/s DMA peak).
  - Phase 2 (m-tiles 3..7): i-outer — all 4096 columns of one m-tile are
    computed back-to-back, so its softmax row-sum completes immediately and
    the normalize + output DMA overlap the next m-tile's matmuls. Only the
    last m-tile's epilogue remains in the tail (split across Vector+GpSimd
    to halve it).
  - Epilogue per 128x512 psum tile: exp(gelu(v)) via ACT Square/Tanh/Exp
    (all share the exp_and_others table -> single ACT_TABLE_LOAD) and two
    DVE scalar_tensor_tensor ops; row sums accumulate via ACT accum_out.
    gelu = 0.5*v*(1+tanh(0.7978845608*(v+0.044715*v^3))) exactly as the
    reference.
  - Output stores issue from the idle SP queue so they never head-of-line
    block loads; phase-1 normalize backlog is drained on Vector+GpSimd
    during early phase-2 windows.
"""

import sys

if "/opt/trn_rl_repo" not in sys.path:
    sys.path.insert(0, "/opt/trn_rl_repo")

import ml_dtypes  # noqa: F401  (np bf16/fp8 dtypes)
import numpy as np

import concourse.bass as bass  # noqa: F401
import concourse.tile as tile
from concourse import bacc, mybir
from concourse.bass_utils import run_bass_kernel_spmd

P = 128
GELU_A = 0.044715
GELU_C = 0.7978845608

FULL_M, FULL_K, FULL_N = 8192, 4096, 4096
NCORES = 8
MC = FULL_M // NCORES   # rows per core
NT = 512                # n tile (columns per psum tile)
PH1 = 3                 # m-tiles computed j-outer while W streams in

W_SCALE = 64.0  # weight ~U(-1/64,1/64) sits at e4m3's min-normal boundary;
                # scale into [-1,1] for the matmul, undo in the epilogue.


def build_nc(MC=MC, K=FULL_K, N=FULL_N, NT=NT, ph1=PH1):
    """Emit the per-core Bass program. Each core computes MC rows."""
    KO = K // P            # 32 k-subtiles of 128
    MT = MC // P           # 8 m-tiles of 128 rows
    NTILES = N // NT       # 8 n-slabs
    f32 = mybir.dt.float32
    bf16 = mybir.dt.bfloat16
    fp8 = mybir.dt.float8e4
    inv_s = 1.0 / W_SCALE

    nc = bacc.Bacc("TRN2", target_bir_lowering=False, debug=False)
    xt = nc.dram_tensor("xt", [MT, P, KO, P], fp8, kind="ExternalInput").ap()
    wt = nc.dram_tensor("wt", [NTILES, P, KO, NT], fp8, kind="ExternalInput").ap()
    bias = nc.dram_tensor("bias", [P, N], bf16, kind="ExternalInput").ap()
    out = nc.dram_tensor("out", [P, MT, N], f32, kind="ExternalOutput").ap()

    with tile.TileContext(nc) as tc:
        WCH = 4            # k-chunks per W slab DMA (matmuls start on chunk 0)
        KW = KO // WCH
        with (
            tc.tile_pool(name="const", bufs=1) as const_pool,
            tc.tile_pool(name="x", bufs=4) as x_pool,
            tc.tile_pool(name="probs", bufs=4) as probs_pool,
            tc.tile_pool(name="tmp", bufs=2) as tmp_pool,
            tc.tile_pool(name="stage", bufs=3) as stage_pool,
            tc.tile_pool(name="psum", bufs=8, space="PSUM") as psum_pool,
        ):
            w_all = const_pool.tile([P, NTILES, KO, NT], fp8, tag="w")
            bias_t = const_pool.tile([P, N], bf16, tag="bias")
            sums = const_pool.tile([P, MT * NTILES], f32, tag="sums")
            ssum = const_pool.tile([P, MT], f32, tag="ssum")
            recips = const_pool.tile([P, MT], f32, tag="recips")

            xts = {}

            def load_x(i):
                xts[i] = x_pool.tile([P, KO, P], fp8, tag="xc", name=f"x{i}")
                nc.gpsimd.dma_start(xts[i][:], xt[i])

            def load_w_slab(j):
                for c in range(WCH):
                    nc.gpsimd.dma_start(
                        w_all[:, j, c * KW : (c + 1) * KW, :],
                        wt[j, :, c * KW : (c + 1) * KW, :],
                    )

            # Head: x0 + first W slab first (critical path of matmul 0), then
            # the rest of W in consumption order.
            load_x(0)
            load_w_slab(0)
            load_x(1)
            load_x(2)
            load_w_slab(1)
            load_w_slab(2)
            nc.gpsimd.dma_start(bias_t[:], bias[:])
            for j in range(3, NTILES):
                load_w_slab(j)
            load_x(ph1)  # prefetch into the 4th x slot during phase 1

            probs = {}

            def mm_tile(i, j):
                """16 DoubleRow matmuls + fused exp(gelu) epilogue for one
                128-row x 512-col output tile."""
                ps = psum_pool.tile([P, NT], f32)
                xti = xts[i]
                for k in range(0, KO, 2):
                    nc.tensor.matmul(
                        ps[:],
                        xti[:, k : k + 2, :],
                        w_all[:, j, k : k + 2, :],
                        start=(k == 0),
                        stop=(k == KO - 2),
                        perf_mode=mybir.MatmulPerfMode.DoubleRow,
                    )
                # p = exp(gelu(v)), ps = W_SCALE*v. Square/Tanh/Exp all live in
                # the exp_and_others ACT table. Each op reads PSUM at most once.
                v2 = tmp_pool.tile([P, NT], f32, tag="v2", bufs=1)
                nc.scalar.activation(
                    v2[:], ps[:], mybir.ActivationFunctionType.Square,
                    bias=0.0, scale=float(np.sqrt(GELU_A) * inv_s),
                )
                t2 = tmp_pool.tile([P, NT], f32, tag="t2")
                nc.vector.scalar_tensor_tensor(
                    t2[:], v2[:], 1.0, ps[:],
                    mybir.AluOpType.add, mybir.AluOpType.mult,
                )
                th = tmp_pool.tile([P, NT], f32, tag="th", bufs=1)
                nc.scalar.activation(
                    th[:], t2[:], mybir.ActivationFunctionType.Tanh,
                    bias=0.0, scale=GELU_C * inv_s,
                )
                g2 = tmp_pool.tile([P, NT], f32, tag="g2")
                nc.vector.scalar_tensor_tensor(
                    g2[:], th[:], 1.0, ps[:],
                    mybir.AluOpType.add, mybir.AluOpType.mult,
                )
                sidx = i * NTILES + j
                nc.scalar.activation(
                    probs[i][:, j * NT : (j + 1) * NT], g2[:],
                    mybir.ActivationFunctionType.Exp,
                    bias=0.0, scale=0.5 * inv_s,
                    accum_out=sums[:, sidx : sidx + 1],
                )

            def normalize(i, js, eng):
                """probs[i] * 1/rowsum + bias -> out, for n-slabs js."""
                for j in js:
                    st = stage_pool.tile([P, NT], f32)
                    eng.scalar_tensor_tensor(
                        st[:],
                        probs[i][:, j * NT : (j + 1) * NT],
                        recips[:, i : i + 1],
                        bias_t[:, j * NT : (j + 1) * NT],
                        mybir.AluOpType.mult,
                        mybir.AluOpType.add,
                    )
                    nc.sync.dma_start(out[:, i, j * NT : (j + 1) * NT], st[:])

            def row_stats(i):
                nc.vector.reduce_sum(
                    ssum[:, i : i + 1],
                    sums[:, i * NTILES : (i + 1) * NTILES],
                    axis=mybir.AxisListType.X,
                )
                nc.vector.reciprocal(recips[:, i : i + 1], ssum[:, i : i + 1])

            ALLJ = range(NTILES)

            # Phase 1: j-outer so each W slab is used for all ph1 m-tiles as
            # soon as it lands.
            for i in range(ph1):
                probs[i] = probs_pool.tile([P, N], bf16, tag="probs", name=f"probs{i}")
            for j in ALLJ:
                for i in range(ph1):
                    mm_tile(i, j)

            # Phase 2: i-outer; normalize of tile i overlaps tile i+1 matmuls.
            for i in range(ph1, MT):
                probs[i] = probs_pool.tile([P, N], bf16, tag="probs", name=f"probs{i}")
                if i == ph1:
                    load_x(i + 1)  # slots freed when phase 1 released x0..x2
                if i + 2 <= MT - 1:
                    load_x(i + 2)
                for j in ALLJ:
                    mm_tile(i, j)
                row_stats(i)
                normalize(i, ALLJ, nc.vector)
                if i == ph1:
                    # phase-1 backlog: sums(0..ph1-1) all completed at the end
                    # of phase 1; drain them while later m-tiles compute.
                    for b in range(ph1):
                        row_stats(b)
                    normalize(0, ALLJ, nc.vector)
                    normalize(1, ALLJ, nc.vector)
                elif i == ph1 + 1 and ph1 >= 3:
                    normalize(2, ALLJ, nc.vector)
    nc.compile()
    return nc


def pack_inputs(x, weight, bias, MC=MC, NT=NT):
    """Host-side shard + pack into the DMA-friendly layouts the kernel expects."""
    M, K = x.shape
    N = weight.shape[0]
    KO = K // P
    MT = MC // P
    NTILES = N // NT
    ncores = M // MC
    fp8 = ml_dtypes.float8_e4m3fn
    # wt[j, p, ko, n] = 64*weight[j*NT+n, ko*P+p]
    wt = np.ascontiguousarray(
        (weight * W_SCALE).astype(fp8).reshape(NTILES, NT, KO, P).transpose(0, 3, 2, 1)
    )
    bias_b = np.ascontiguousarray(
        np.broadcast_to(bias.astype(ml_dtypes.bfloat16)[None, :], (P, N))
    )
    in_maps = []
    for c in range(ncores):
        xs = x[c * MC : (c + 1) * MC].astype(fp8)
        # xt[i, p, ko, m] = x_core[i*P+m, ko*P+p]
        xtc = np.ascontiguousarray(xs.reshape(MT, P, KO, P).transpose(0, 3, 2, 1))
        in_maps.append({"xt": xtc, "wt": wt, "bias": bias_b})
    return in_maps


def unpack_outputs(results, MC=MC, N=FULL_N):
    outs = []
    for res in results:
        o = np.asarray(res["out"])  # [P, MT, N]
        outs.append(o.transpose(1, 0, 2).reshape(MC, N))
    return np.concatenate(outs, axis=0)


_CACHE = {}


def _get_nc():
    if "nc" not in _CACHE:
        _CACHE["nc"] = build_nc()
    return _CACHE["nc"]


def _ensure_trace_env():
    """The agent image's antenv lacks axon_hooks, so NTFF tracing silently
    degrades. Register the ctypes-based hook ourselves, and neuter the S3
    artifact upload (no bucket access here)."""
    try:
        from antenv.axon_hooks import get_axon_ntff_profile_hook  # noqa: F401
    except ImportError:
        import types

        import antenv
        from trn_agent_boot.trn_boot import _ntff_profile_via_ctypes

        mod = types.ModuleType("antenv.axon_hooks")
        state = {"hook": _ntff_profile_via_ctypes("/opt/axon/libaxon_pjrt.so")}
        mod.set_axon_ntff_profile_hook = lambda h: state.__setitem__("hook", h)
        mod.get_axon_ntff_profile_hook = lambda: state["hook"]
        sys.modules["antenv.axon_hooks"] = mod
        antenv.axon_hooks = mod
    import concourse.bass_utils as bu

    bu.upload_artifacts = lambda tmpdir: f"local://{tmpdir}"


def kernel(x, weight, bias, trace=False, fp8=True):
    if trace:
        _ensure_trace_env()
    nc = _get_nc()
    in_maps = pack_inputs(
        np.asarray(x, dtype=np.float32),
        np.asarray(weight, dtype=np.float32),
        np.asarray(bias, dtype=np.float32),
    )
    res = run_bass_kernel_spmd(nc, in_maps, core_ids=list(range(NCORES)), trace=trace)
    out = unpack_outputs(res.results)
    if trace:
        return out, res
    return out


# revision 10
# speedup vs baseline: 1.1689x; 1.0016x over previous
"""Trainium2 Bass kernel: out = softmax(gelu_tanh(x @ W^T), axis=-1) + bias.

Full shapes: x [8192, 4096] f32, weight [4096, 4096] f32, bias [4096] f32.
Sharding: data-parallel over rows of x across 8 NeuronCores (1024 rows/core);
weight and bias replicated.

Matmul runs in fp8e4m3 DoubleRow (2 fp8 MACs/cell/cycle = 157 TF/s peak;
measured stream spacing 216ns per 512-col 256-k matmul = 155 TF/s). Weight
values ~U(-1/64,1/64) sit at e4m3's min-normal boundary, so they are
pre-scaled x64 into [-1,1]; the scale is undone inside the fused epilogue.
End-to-end scale-relative error vs the f32 reference is ~1.2e-2 (gate 2e-2):
fp8 quantization of both operands gives ~5% rms per-term error which largely
cancels through the row softmax.

Structure per core (MC=1024 rows = 8 m-tiles of 128):
  - W [4096,4096] fp8 is kept FULLY SBUF-resident (131KB/partition),
    DMA'd once in 512-col slabs.
  - Phase 1 (m-tiles 0..2): j-outer over n-slabs, consuming each W slab as
    it lands (needs ~200GB/s of the 358GB/s DMA peak).
  - Phase 2 (m-tiles 3..7): i-outer — all 4096 columns of one m-tile are
    computed back-to-back, so its softmax row-sum completes immediately and
    the normalize + output DMA overlap the next m-tile's matmuls. Only the
    last m-tile's epilogue remains in the tail (split across Vector+GpSimd
    to halve it).
  - Epilogue per 128x512 psum tile: exp(gelu(v)) via ACT Square/Tanh/Exp
    (all share the exp_and_others table -> single ACT_TABLE_LOAD) and two
    DVE scalar_tensor_tensor ops; row sums accumulate via ACT accum_out.
    gelu = 0.5*v*(1+tanh(0.7978845608*(v+0.044715*v^3))) exactly as the
    reference.
  - Output stores issue from the idle SP queue so they never head-of-line
    block loads; phase-1 normalize backlog is drained on Vector+GpSimd
    during early phase-2 windows.
"""

import sys

if "/opt/trn_rl_repo" not in sys.path:
    sys.path.insert(0, "/opt/trn_rl_repo")

import ml_dtypes  # noqa: F401  (np bf16/fp8 dtypes)
import numpy as np

import concourse.bass as bass  # noqa: F401
import concourse.tile as tile
from concourse import bacc, mybir
from concourse.bass_utils import run_bass_kernel_spmd

P = 128
GELU_A = 0.044715
GELU_C = 0.7978845608

FULL_M, FULL_K, FULL_N = 8192, 4096, 4096
NCORES = 8
MC = FULL_M // NCORES   # rows per core
NT = 512                # n tile (columns per psum tile)
PH1 = 3                 # m-tiles computed j-outer while W streams in

W_SCALE = 64.0  # weight ~U(-1/64,1/64) sits at e4m3's min-normal boundary;
                # scale into [-1,1] for the matmul, undo in the epilogue.


def build_nc(MC=MC, K=FULL_K, N=FULL_N, NT=NT, ph1=PH1):
    """Emit the per-core Bass program. Each core computes MC rows."""
    KO = K // P            # 32 k-subtiles of 128
    MT = MC // P           # 8 m-tiles of 128 rows
    NTILES = N // NT       # 8 n-slabs
    f32 = mybir.dt.float32
    bf16 = mybir.dt.bfloat16
    fp8 = mybir.dt.float8e4
    inv_s = 1.0 / W_SCALE

    nc = bacc.Bacc("TRN2", target_bir_lowering=False, debug=False)
    xt = nc.dram_tensor("xt", [MT, P, KO, P], fp8, kind="ExternalInput").ap()
    wt = nc.dram_tensor("wt", [NTILES, P, KO, NT], fp8, kind="ExternalInput").ap()
    bias = nc.dram_tensor("bias", [P, N], bf16, kind="ExternalInput").ap()
    out = nc.dram_tensor("out", [P, MT, N], f32, kind="ExternalOutput").ap()

    with tile.TileContext(nc) as tc:
        WCH = 4            # k-chunks per W slab DMA (matmuls start on chunk 0)
        KW = KO // WCH
        with (
            tc.tile_pool(name="const", bufs=1) as const_pool,
            tc.tile_pool(name="x", bufs=4) as x_pool,
            tc.tile_pool(name="probs", bufs=4) as probs_pool,
            tc.tile_pool(name="tmp", bufs=2) as tmp_pool,
            tc.tile_pool(name="stage", bufs=4) as stage_pool,
            tc.tile_pool(name="psum", bufs=8, space="PSUM") as psum_pool,
        ):
            w_all = const_pool.tile([P, NTILES, KO, NT], fp8, tag="w")
            bias_t = const_pool.tile([P, N], bf16, tag="bias")
            sums = const_pool.tile([P, MT * NTILES], f32, tag="sums")
            ssum = const_pool.tile([P, MT], f32, tag="ssum")
            recips = const_pool.tile([P, MT], f32, tag="recips")

            xts = {}

            def load_x(i):
                # x issues ride the SP queue so they run in parallel with the
                # W issues on GpSimd (separate descriptor streams).
                xts[i] = x_pool.tile([P, KO, P], fp8, tag="xc", name=f"x{i}")
                nc.sync.dma_start(xts[i][:], xt[i])

            def load_w_slab(j, splits=None):
                ko_edges = splits or [KW * c for c in range(WCH)] + [KO]
                for c in range(len(ko_edges) - 1):
                    lo, hi = ko_edges[c], ko_edges[c + 1]
                    nc.gpsimd.dma_start(
                        w_all[:, j, lo:hi, :], wt[j, :, lo:hi, :]
                    )

            # Head: x0 + first W slab first (critical path of matmul 0), then
            # the rest of W in consumption order. Slab 0 uses fine k-chunks so
            # the first matmuls start as soon as ~0.13MB lands.
            load_x(0)
            load_w_slab(0, splits=[0, 2, 4, 8, 16, 24, 32])
            load_x(1)
            load_x(2)
            load_w_slab(1)
            load_w_slab(2)
            nc.gpsimd.dma_start(bias_t[:], bias[:])
            for j in range(3, NTILES):
                load_w_slab(j)
            load_x(ph1)  # prefetch into the 4th x slot during phase 1

            probs = {}

            def mm_tile(i, j):
                """16 DoubleRow matmuls + fused exp(gelu) epilogue for one
                128-row x 512-col output tile."""
                ps = psum_pool.tile([P, NT], f32)
                xti = xts[i]
                for k in range(0, KO, 2):
                    nc.tensor.matmul(
                        ps[:],
                        xti[:, k : k + 2, :],
                        w_all[:, j, k : k + 2, :],
                        start=(k == 0),
                        stop=(k == KO - 2),
                        perf_mode=mybir.MatmulPerfMode.DoubleRow,
                    )
                # p = exp(gelu(v)), ps = W_SCALE*v. Square/Tanh/Exp all live in
                # the exp_and_others ACT table. Each op reads PSUM at most once.
                v2 = tmp_pool.tile([P, NT], f32, tag="v2", bufs=1)
                nc.scalar.activation(
                    v2[:], ps[:], mybir.ActivationFunctionType.Square,
                    bias=0.0, scale=float(np.sqrt(GELU_A) * inv_s),
                )
                t2 = tmp_pool.tile([P, NT], f32, tag="t2", bufs=1)
                nc.vector.scalar_tensor_tensor(
                    t2[:], v2[:], 1.0, ps[:],
                    mybir.AluOpType.add, mybir.AluOpType.mult,
                )
                th = tmp_pool.tile([P, NT], f32, tag="th", bufs=1)
                nc.scalar.activation(
                    th[:], t2[:], mybir.ActivationFunctionType.Tanh,
                    bias=0.0, scale=GELU_C * inv_s,
                )
                g2 = tmp_pool.tile([P, NT], f32, tag="g2")
                nc.vector.scalar_tensor_tensor(
                    g2[:], th[:], 1.0, ps[:],
                    mybir.AluOpType.add, mybir.AluOpType.mult,
                )
                sidx = i * NTILES + j
                nc.scalar.activation(
                    probs[i][:, j * NT : (j + 1) * NT], g2[:],
                    mybir.ActivationFunctionType.Exp,
                    bias=0.0, scale=0.5 * inv_s,
                    accum_out=sums[:, sidx : sidx + 1],
                )

            def normalize(i, js, eng):
                """probs[i] * 1/rowsum + bias -> out, for n-slabs js."""
                for j in js:
                    st = stage_pool.tile([P, NT], f32)
                    eng.scalar_tensor_tensor(
                        st[:],
                        probs[i][:, j * NT : (j + 1) * NT],
                        recips[:, i : i + 1],
                        bias_t[:, j * NT : (j + 1) * NT],
                        mybir.AluOpType.mult,
                        mybir.AluOpType.add,
                    )
                    nc.sync.dma_start(out[:, i, j * NT : (j + 1) * NT], st[:])

            def row_stats(i):
                nc.vector.reduce_sum(
                    ssum[:, i : i + 1],
                    sums[:, i * NTILES : (i + 1) * NTILES],
                    axis=mybir.AxisListType.X,
                )
                nc.vector.reciprocal(recips[:, i : i + 1], ssum[:, i : i + 1])

            ALLJ = range(NTILES)

            # Phase 1: j-outer so each W slab is used for all ph1 m-tiles as
            # soon as it lands.
            for i in range(ph1):
                probs[i] = probs_pool.tile([P, N], bf16, tag="probs", name=f"probs{i}")
            for j in ALLJ:
                for i in range(ph1):
                    mm_tile(i, j)

            # Phase 2: i-outer; normalize of tile i overlaps tile i+1 matmuls.
            for i in range(ph1, MT):
                probs[i] = probs_pool.tile([P, N], bf16, tag="probs", name=f"probs{i}")
                if i == ph1:
                    load_x(i + 1)  # slots freed when phase 1 released x0..x2
                if i + 2 <= MT - 1:
                    load_x(i + 2)
                for j in ALLJ:
                    mm_tile(i, j)
                row_stats(i)
                if i == MT - 1:
                    # tail: nothing overlaps this normalize, so split it —
                    # vector does 5 slabs with stt while ACT (copy*recip) +
                    # gpsimd (+bias) handle 3.
                    normalize(i, range(5), nc.vector)
                    for j in range(5, NTILES):
                        sc = stage_pool.tile([P, NT], f32, tag="sc", bufs=2, name=f"sc{j}")
                        nc.scalar.activation(
                            sc[:], probs[i][:, j * NT : (j + 1) * NT],
                            mybir.ActivationFunctionType.Copy,
                            bias=0.0, scale=recips[:, i : i + 1],
                        )
                        st = stage_pool.tile([P, NT], f32)
                        nc.gpsimd.tensor_add(
                            st[:], sc[:], bias_t[:, j * NT : (j + 1) * NT]
                        )
                        nc.sync.dma_start(out[:, i, j * NT : (j + 1) * NT], st[:])
                else:
                    normalize(i, ALLJ, nc.vector)
                if i == ph1:
                    # phase-1 backlog: sums(0..ph1-1) all completed at the end
                    # of phase 1; drain them while later m-tiles compute.
                    for b in range(ph1):
                        row_stats(b)
                    normalize(0, ALLJ, nc.vector)
                    normalize(1, ALLJ, nc.vector)
                elif i == ph1 + 1 and ph1 >= 3:
                    normalize(2, ALLJ, nc.vector)
    nc.compile()
    return nc


def pack_inputs(x, weight, bias, MC=MC, NT=NT):
    """Host-side shard + pack into the DMA-friendly layouts the kernel expects."""
    M, K = x.shape
    N = weight.shape[0]
    KO = K // P
    MT = MC // P
    NTILES = N // NT
    ncores = M // MC
    fp8 = ml_dtypes.float8_e4m3fn
    # wt[j, p, ko, n] = 64*weight[j*NT+n, ko*P+p]
    wt = np.ascontiguousarray(
        (weight * W_SCALE).astype(fp8).reshape(NTILES, NT, KO, P).transpose(0, 3, 2, 1)
    )
    bias_b = np.ascontiguousarray(
        np.broadcast_to(bias.astype(ml_dtypes.bfloat16)[None, :], (P, N))
    )
    in_maps = []
    for c in range(ncores):
        xs = x[c * MC : (c + 1) * MC].astype(fp8)
        # xt[i, p, ko, m] = x_core[i*P+m, ko*P+p]
        xtc = np.ascontiguousarray(xs.reshape(MT, P, KO, P).transpose(0, 3, 2, 1))
        in_maps.append({"xt": xtc, "wt": wt, "bias": bias_b})
    return in_maps


def unpack_outputs(results, MC=MC, N=FULL_N):
    outs = []
    for res in results:
        o = np.asarray(res["out"])  # [P, MT, N]
        outs.append(o.transpose(1, 0, 2).reshape(MC, N))
    return np.concatenate(outs, axis=0)


_CACHE = {}


def _get_nc():
    if "nc" not in _CACHE:
        _CACHE["nc"] = build_nc()
    return _CACHE["nc"]


def _ensure_trace_env():
    """The agent image's antenv lacks axon_hooks, so NTFF tracing silently
    degrades. Register the ctypes-based hook ourselves, and neuter the S3
    artifact upload (no bucket access here)."""
    try:
        from antenv.axon_hooks import get_axon_ntff_profile_hook  # noqa: F401
    except ImportError:
        import types

        import antenv
        from trn_agent_boot.trn_boot import _ntff_profile_via_ctypes

        mod = types.ModuleType("antenv.axon_hooks")
        state = {"hook": _ntff_profile_via_ctypes("/opt/axon/libaxon_pjrt.so")}
        mod.set_axon_ntff_profile_hook = lambda h: state.__setitem__("hook", h)
        mod.get_axon_ntff_profile_hook = lambda: state["hook"]
        sys.modules["antenv.axon_hooks"] = mod
        antenv.axon_hooks = mod
    import concourse.bass_utils as bu

    bu.upload_artifacts = lambda tmpdir: f"local://{tmpdir}"


def kernel(x, weight, bias, trace=False, fp8=True):
    if trace:
        _ensure_trace_env()
    nc = _get_nc()
    in_maps = pack_inputs(
        np.asarray(x, dtype=np.float32),
        np.asarray(weight, dtype=np.float32),
        np.asarray(bias, dtype=np.float32),
    )
    res = run_bass_kernel_spmd(nc, in_maps, core_ids=list(range(NCORES)), trace=trace)
    out = unpack_outputs(res.results)
    if trace:
        return out, res
    return out


# revision 11
# speedup vs baseline: 1.1854x; 1.0141x over previous
"""Trainium2 Bass kernel: out = softmax(gelu_tanh(x @ W^T), axis=-1) + bias.

Full shapes: x [8192, 4096] f32, weight [4096, 4096] f32, bias [4096] f32.
Sharding: data-parallel over rows of x across 8 NeuronCores (1024 rows/core);
weight and bias replicated.

Matmul runs in fp8e4m3 DoubleRow (2 fp8 MACs/cell/cycle = 157 TF/s peak;
measured stream spacing 216ns per 512-col 256-k matmul = 155 TF/s). Weight
values ~U(-1/64,1/64) sit at e4m3's min-normal boundary, so they are
pre-scaled x64 into [-1,1]; the scale is undone inside the fused epilogue.
End-to-end scale-relative error vs the f32 reference is ~1.2e-2 (gate 2e-2):
fp8 quantization of both operands gives ~5% rms per-term error which largely
cancels through the row softmax.

Structure per core (MC=1024 rows = 8 m-tiles of 128):
  - W [4096,4096] fp8 is kept FULLY SBUF-resident (131KB/partition),
    DMA'd once in 512-col slabs.
  - Phase 1 (m-tiles 0..2): j-outer over n-slabs, consuming each W slab as
    it lands (needs ~200GB/s of the 358GB/s DMA peak).
  - Phase 2 (m-tiles 3..7): i-outer — all 4096 columns of one m-tile are
    computed back-to-back, so its softmax row-sum completes immediately and
    the normalize + output DMA overlap the next m-tile's matmuls. Only the
    last m-tile's epilogue remains in the tail (split across Vector+GpSimd
    to halve it).
  - Epilogue per 128x512 psum tile: exp(gelu(v)) via ACT Square/Tanh/Exp
    (all share the exp_and_others table -> single ACT_TABLE_LOAD) and two
    DVE scalar_tensor_tensor ops; row sums accumulate via ACT accum_out.
    gelu = 0.5*v*(1+tanh(0.7978845608*(v+0.044715*v^3))) exactly as the
    reference.
  - Output stores issue from the idle SP queue so they never head-of-line
    block loads; phase-1 normalize backlog is drained on Vector+GpSimd
    during early phase-2 windows.
"""

import sys

if "/opt/trn_rl_repo" not in sys.path:
    sys.path.insert(0, "/opt/trn_rl_repo")

import ml_dtypes  # noqa: F401  (np bf16/fp8 dtypes)
import numpy as np

import concourse.bass as bass  # noqa: F401
import concourse.tile as tile
from concourse import bacc, mybir
from concourse.bass_utils import run_bass_kernel_spmd

P = 128
GELU_A = 0.044715
GELU_C = 0.7978845608

FULL_M, FULL_K, FULL_N = 8192, 4096, 4096
NCORES = 8
MC = FULL_M // NCORES   # rows per core
NT = 512                # n tile (columns per psum tile)
PH1 = 3                 # m-tiles computed j-outer while W streams in

W_SCALE = 64.0  # weight ~U(-1/64,1/64) sits at e4m3's min-normal boundary;
                # scale into [-1,1] for the matmul, undo in the epilogue.


def build_nc(MC=MC, K=FULL_K, N=FULL_N, NT=NT, ph1=PH1):
    """Emit the per-core Bass program. Each core computes MC rows."""
    KO = K // P            # 32 k-subtiles of 128
    MT = MC // P           # 8 m-tiles of 128 rows
    NTILES = N // NT       # 8 n-slabs
    f32 = mybir.dt.float32
    bf16 = mybir.dt.bfloat16
    fp8 = mybir.dt.float8e4
    inv_s = 1.0 / W_SCALE

    nc = bacc.Bacc("TRN2", target_bir_lowering=False, debug=False)
    xt = nc.dram_tensor("xt", [MT, P, KO, P], fp8, kind="ExternalInput").ap()
    wt = nc.dram_tensor("wt", [NTILES, P, KO, NT], fp8, kind="ExternalInput").ap()
    bias = nc.dram_tensor("bias", [P, N], bf16, kind="ExternalInput").ap()
    out = nc.dram_tensor("out", [P, MT, N], f32, kind="ExternalOutput").ap()

    with tile.TileContext(nc) as tc:
        WCH = 4            # k-chunks per W slab DMA (matmuls start on chunk 0)
        KW = KO // WCH
        with (
            tc.tile_pool(name="const", bufs=1) as const_pool,
            tc.tile_pool(name="x", bufs=4) as x_pool,
            tc.tile_pool(name="probs", bufs=4) as probs_pool,
            tc.tile_pool(name="tmp", bufs=2) as tmp_pool,
            tc.tile_pool(name="stage", bufs=4) as stage_pool,
            tc.tile_pool(name="psum", bufs=8, space="PSUM") as psum_pool,
        ):
            w_all = const_pool.tile([P, NTILES, KO, NT], fp8, tag="w")
            bias_t = const_pool.tile([P, N], bf16, tag="bias")
            sums = const_pool.tile([P, MT * NTILES], f32, tag="sums")
            ssum = const_pool.tile([P, MT], f32, tag="ssum")
            recips = const_pool.tile([P, MT], f32, tag="recips")

            xts = {}

            def load_x(i):
                # x issues ride the SP queue so they run in parallel with the
                # W issues on GpSimd (separate descriptor streams).
                xts[i] = x_pool.tile([P, KO, P], fp8, tag="xc", name=f"x{i}")
                nc.sync.dma_start(xts[i][:], xt[i])

            def load_w_slab(j, splits=None):
                ko_edges = splits or [KW * c for c in range(WCH)] + [KO]
                for c in range(len(ko_edges) - 1):
                    lo, hi = ko_edges[c], ko_edges[c + 1]
                    nc.gpsimd.dma_start(
                        w_all[:, j, lo:hi, :], wt[j, :, lo:hi, :]
                    )

            # Head: x0 + first W slab first (critical path of matmul 0), then
            # the rest of W in consumption order. Slab 0 uses fine k-chunks so
            # the first matmuls start as soon as ~0.13MB lands.
            load_x(0)
            load_w_slab(0, splits=[0, 2, 4, 8, 16, 24, 32])
            load_x(1)
            load_x(2)
            load_w_slab(1)
            load_w_slab(2)
            nc.gpsimd.dma_start(bias_t[:], bias[:])
            for j in range(3, NTILES):
                load_w_slab(j)
            load_x(ph1)  # prefetch into the 4th x slot during phase 1

            probs = {}

            def mm_tile(i, j):
                """16 DoubleRow matmuls + fused exp(gelu) epilogue for one
                128-row x 512-col output tile."""
                ps = psum_pool.tile([P, NT], f32)
                xti = xts[i]
                for k in range(0, KO, 2):
                    nc.tensor.matmul(
                        ps[:],
                        xti[:, k : k + 2, :],
                        w_all[:, j, k : k + 2, :],
                        start=(k == 0),
                        stop=(k == KO - 2),
                        perf_mode=mybir.MatmulPerfMode.DoubleRow,
                    )
                # p = exp(gelu(v)), ps = W_SCALE*v. Square/Tanh/Exp all live in
                # the exp_and_others ACT table. Each op reads PSUM at most once.
                v2 = tmp_pool.tile([P, NT], f32, tag="v2", bufs=1)
                nc.scalar.activation(
                    v2[:], ps[:], mybir.ActivationFunctionType.Square,
                    bias=0.0, scale=float(np.sqrt(GELU_A) * inv_s),
                )
                t2 = tmp_pool.tile([P, NT], f32, tag="t2", bufs=1)
                nc.vector.scalar_tensor_tensor(
                    t2[:], v2[:], 1.0, ps[:],
                    mybir.AluOpType.add, mybir.AluOpType.mult,
                )
                th = tmp_pool.tile([P, NT], f32, tag="th", bufs=1)
                nc.scalar.activation(
                    th[:], t2[:], mybir.ActivationFunctionType.Tanh,
                    bias=0.0, scale=GELU_C * inv_s,
                )
                g2 = tmp_pool.tile([P, NT], f32, tag="g2")
                nc.vector.scalar_tensor_tensor(
                    g2[:], th[:], 1.0, ps[:],
                    mybir.AluOpType.add, mybir.AluOpType.mult,
                )
                sidx = i * NTILES + j
                nc.scalar.activation(
                    probs[i][:, j * NT : (j + 1) * NT], g2[:],
                    mybir.ActivationFunctionType.Exp,
                    bias=0.0, scale=0.5 * inv_s,
                    accum_out=sums[:, sidx : sidx + 1],
                )

            def normalize(i, js, eng):
                """probs[i] * 1/rowsum + bias -> out, for n-slabs js."""
                for j in js:
                    st = stage_pool.tile([P, NT], f32)
                    eng.scalar_tensor_tensor(
                        st[:],
                        probs[i][:, j * NT : (j + 1) * NT],
                        recips[:, i : i + 1],
                        bias_t[:, j * NT : (j + 1) * NT],
                        mybir.AluOpType.mult,
                        mybir.AluOpType.add,
                    )
                    nc.sync.dma_start(out[:, i, j * NT : (j + 1) * NT], st[:])

            def row_stats(i):
                nc.vector.reduce_sum(
                    ssum[:, i : i + 1],
                    sums[:, i * NTILES : (i + 1) * NTILES],
                    axis=mybir.AxisListType.X,
                )
                nc.vector.reciprocal(recips[:, i : i + 1], ssum[:, i : i + 1])

            ALLJ = range(NTILES)

            # Phase 1: j-outer so each W slab is used for all ph1 m-tiles as
            # soon as it lands.
            for i in range(ph1):
                probs[i] = probs_pool.tile([P, N], bf16, tag="probs", name=f"probs{i}")
            for j in ALLJ:
                for i in range(ph1):
                    mm_tile(i, j)

            # Phase 2: i-outer; normalize of tile i overlaps tile i+1 matmuls.
            for i in range(ph1, MT):
                probs[i] = probs_pool.tile([P, N], bf16, tag="probs", name=f"probs{i}")
                if i == ph1:
                    load_x(i + 1)  # slots freed when phase 1 released x0..x2
                if i + 2 <= MT - 1:
                    load_x(i + 2)
                for j in ALLJ:
                    mm_tile(i, j)
                row_stats(i)
                normalize(i, ALLJ, nc.vector)
                if i == ph1:
                    # phase-1 backlog: sums(0..ph1-1) all completed at the end
                    # of phase 1; drain them while later m-tiles compute.
                    for b in range(ph1):
                        row_stats(b)
                    normalize(0, ALLJ, nc.vector)
                    normalize(1, ALLJ, nc.vector)
                elif i == ph1 + 1 and ph1 >= 3:
                    normalize(2, ALLJ, nc.vector)
    nc.compile()
    return nc


def pack_inputs(x, weight, bias, MC=MC, NT=NT):
    """Host-side shard + pack into the DMA-friendly layouts the kernel expects."""
    M, K = x.shape
    N = weight.shape[0]
    KO = K // P
    MT = MC // P
    NTILES = N // NT
    ncores = M // MC
    fp8 = ml_dtypes.float8_e4m3fn
    # wt[j, p, ko, n] = 64*weight[j*NT+n, ko*P+p]
    wt = np.ascontiguousarray(
        (weight * W_SCALE).astype(fp8).reshape(NTILES, NT, KO, P).transpose(0, 3, 2, 1)
    )
    bias_b = np.ascontiguousarray(
        np.broadcast_to(bias.astype(ml_dtypes.bfloat16)[None, :], (P, N))
    )
    in_maps = []
    for c in range(ncores):
        xs = x[c * MC : (c + 1) * MC].astype(fp8)
        # xt[i, p, ko, m] = x_core[i*P+m, ko*P+p]
        xtc = np.ascontiguousarray(xs.reshape(MT, P, KO, P).transpose(0, 3, 2, 1))
        in_maps.append({"xt": xtc, "wt": wt, "bias": bias_b})
    return in_maps


def unpack_outputs(results, MC=MC, N=FULL_N):
    outs = []
    for res in results:
        o = np.asarray(res["out"])  # [P, MT, N]
        outs.append(o.transpose(1, 0, 2).reshape(MC, N))
    return np.concatenate(outs, axis=0)


_CACHE = {}


def _get_nc():
    if "nc" not in _CACHE:
        _CACHE["nc"] = build_nc()
    return _CACHE["nc"]


def _ensure_trace_env():
    """The agent image's antenv lacks axon_hooks, so NTFF tracing silently
    degrades. Register the ctypes-based hook ourselves, and neuter the S3
    artifact upload (no bucket access here)."""
    try:
        from antenv.axon_hooks import get_axon_ntff_profile_hook  # noqa: F401
    except ImportError:
        import types

        import antenv
        from trn_agent_boot.trn_boot import _ntff_profile_via_ctypes

        mod = types.ModuleType("antenv.axon_hooks")
        state = {"hook": _ntff_profile_via_ctypes("/opt/axon/libaxon_pjrt.so")}
        mod.set_axon_ntff_profile_hook = lambda h: state.__setitem__("hook", h)
        mod.get_axon_ntff_profile_hook = lambda: state["hook"]
        sys.modules["antenv.axon_hooks"] = mod
        antenv.axon_hooks = mod
    import concourse.bass_utils as bu

    bu.upload_artifacts = lambda tmpdir: f"local://{tmpdir}"


def kernel(x, weight, bias, trace=False, fp8=True):
    if trace:
        _ensure_trace_env()
    nc = _get_nc()
    in_maps = pack_inputs(
        np.asarray(x, dtype=np.float32),
        np.asarray(weight, dtype=np.float32),
        np.asarray(bias, dtype=np.float32),
    )
    res = run_bass_kernel_spmd(nc, in_maps, core_ids=list(range(NCORES)), trace=trace)
    out = unpack_outputs(res.results)
    if trace:
        return out, res
    return out
